# revision 49
# baseline (speedup 1.0000x reference)
"""Trainium2 Bass kernel for nn_BackwardTransformLayer (inverse DWT synthesis step).

Math: out[r, 2j+s] = sum_{p=0..3} g[2p+s]*d[r,(j+p+s')%M] + h[2p+s]*a[r,...]
  (g = flip(scaling) with odd idx negated; h = scaling; even outputs use
   shifts 0..3 of taps g[0,2,4,6], odd outputs shifts 1..4 of g[1,3,5,7])

Strategy (8 cores data-parallel over rows, 512 rows/core):
  - fp16 I/O: inputs are quantized to fp16 on the host and the output is
    written fp16, halving HBM traffic (f32 I/O saturates the ~358 GB/s/core
    DMA ceiling at ~183us; fp16 floor is ~94us). Quantization noise is
    ~3e-4 fro rel err, far below the 2e-2 gate. fp16 also halves PE
    transpose cost (1 cyc/row vs 2 for fp32).
  - The polyphase stencil along columns is a banded linear operator: for each
    128-column input block k, out[:, 256k:256k+256] = d_blk @ W_d + a_blk @ W_a
    plus a "halo" contribution from the first 4 columns of block k+1
    (circularly wrapped) hitting output columns 249..255 of the chunk.
  - TensorE transposes input blocks (8 blocks -> one [128,1024] PSUM bank),
    DVE copies them to SBUF, and TensorE computes the banded products with
    stationary = dT slice and moving = W[incol, outcol]; results land
    naturally oriented [row, outcol] in PSUM.
  - Halo: the 4 wrapped columns per block are gathered (strided stationary)
    into one PE transpose per (stream, half), then a single [128k x 224]
    matmul per half accumulates both streams' halo contributions; one strided
    DVE add folds them into output columns 249..255 of every chunk. (The old
    path - 64 tiny strided MACs per group - is kept behind BASS_HALO=stt.)
  - Output chunk-pair copies PSUM->SBUF rotate ACT/ACT/DVE/ACT (only ACT and
    DVE may read PSUM on HW; GPSIMD raises a BIR verifier error).
  - Scheduling (sim-driven): output stores issue on the GPSIMD HWDGE queue
    right after the DVE halo add they depend on (in-order queues: a store
    waiting at an engine's queue head must not block compute or loads);
    input loads keep the sync queue to themselves; h1 stores of groups 0..2
    are DEFERRED and re-issued on the then-idle sync queue during the final
    group so the DMA engines stay fed through the PE-paced drain; group 0's
    first strips are split so the first transposes start ~2us in; the
    wrapped first-4-columns are copied SBUF->SBUF by DVE late in each group
    (emitting at load time would clog the 4-deep DVE wait queue).
  - TimelineSim (simprof.py) estimate: ~111us; per-core busy: DMA 94us
    (33.5 MB at ~360 GB/s, the roofline), PE 86us, ACT ~65us, DVE ~65us.

Env:
  BASS_IO16=1 (default) fp16 DRAM I/O + fp16 matmuls; =0 f32 I/O (legacy).
  BASS_MM_F32R=1 (default) legacy-path matmuls in float32r; =0 exact fp32.
  BASS_HALO=mm|stt (default mm) halo via batched matmul or strided MACs.
  BASS_OUT_PATTERN=ssvs  output-copy engine rotation: s=ACT v=DVE.
  BASS_OCTET=1 (default) 8-block [128,1024] transpose tiles; =0 quads.
  BASS_STORE_ENG=pool|sync|scalar (default pool) store HWDGE queue.
"""

import os
import sys
from contextlib import ExitStack

import numpy as np

sys.path.insert(0, "/opt/trn_rl_repo")

import concourse.bass as bass  # noqa: E402
import concourse.mybir as mybir  # noqa: E402
import concourse.tile as tile  # noqa: E402
from concourse import bacc  # noqa: E402
from concourse.bass_utils import run_bass_kernel_spmd  # noqa: E402

N_CORES = 8
N_ROWS = 4096
M = 8192  # input columns per row
PG = 128  # rows per group (partition dim)
BLK = 128  # input columns per block
OUTW = 2 * BLK  # output columns per chunk
HALF = M // 2  # input columns per half-strip
NBLK_HALF = HALF // BLK  # 32 blocks per half-strip
NBLK = M // BLK  # 64 blocks
EXTW = HALF + BLK  # extended strip width (one extra block; 4 cols used)
F32 = mybir.dt.float32
F32R = mybir.dt.float32r
F16 = mybir.dt.float16

IO16 = os.environ.get("BASS_IO16", "1") == "1"
MM_F32R = os.environ.get("BASS_MM_F32R", "1") == "1"
HALO_MM = os.environ.get("BASS_HALO", "mm") == "mm"
# chunk-pair copy engines: only ACT ('s') and DVE ('v') may read PSUM on
# real HW (GPSIMD raises "cannot access PSUM" in the BIR verifier)
OUT_PATTERN = os.environ.get("BASS_OUT_PATTERN", "ssvs")
OCTET = os.environ.get("BASS_OCTET", "1") == "1"

NP_IO = np.float16 if IO16 else np.float32

_BUILD_CACHE = {}


def _halo_positions():
    """Static (stream, kp, n, tap) positions of halo coefficients.

    Chunk outcol n (0..255) gets a contribution coeff[tap] * x[:, 128*(k+1)+kp]
    from the next block's first 4 columns.
    """
    pos = []
    for sti in range(2):  # 0 = details (g), 1 = approximation (h)
        for v in range(128):
            for s in range(4):
                kp = v + s - 128
                if 0 <= kp <= 3:
                    pos.append((sti, kp, 2 * v, 2 * s))
                kp2 = v + 1 + s - 128
                if 0 <= kp2 <= 3:
                    pos.append((sti, kp2, 2 * v + 1, 2 * s + 1))
    return pos


HALO_POS = _halo_positions()  # 32 entries
HALO_NLO = 249  # lowest chunk outcol touched by halo
HALO_W = 256 - HALO_NLO  # 7 consecutive outcols per chunk
HALO_NC = 16  # chunks per halo batch (quarter of a group)


def _build_weights(scaling: np.ndarray):
    h = np.asarray(scaling, dtype=np.float32)
    g = h[::-1].copy()
    g[1::2] *= -1.0

    def build_main(f):
        W = np.zeros((BLK, OUTW), np.float32)
        for k in range(BLK):
            for v in range(BLK):
                s = k - v
                if 0 <= s <= 3:
                    W[k, 2 * v] = f[2 * s]
                s = k - v - 1
                if 0 <= s <= 3:
                    W[k, 2 * v + 1] = f[2 * s + 1]
        return W

    hvec = np.zeros((128, len(HALO_POS)), np.float32)
    for i, (sti, kp, n, tap) in enumerate(HALO_POS):
        hvec[:, i] = (g if sti == 0 else h)[tap]

    def build_halo(f, sti_sel):
        # row = 4c + kp (transposed gather layout), col = 7c + (n - 249);
        # one quarter-group (16 chunks) worth - identical for every quarter
        W = np.zeros((4 * HALO_NC, HALO_W * HALO_NC), np.float32)
        for sti, kp, n, tap in HALO_POS:
            if sti != sti_sel:
                continue
            for c in range(HALO_NC):
                W[4 * c + kp, HALO_W * c + (n - HALO_NLO)] = f[tap]
        return W

    return build_main(g), build_main(h), hvec, build_halo(g, 0), build_halo(h, 1)


def host_consts(scaling: np.ndarray) -> dict:
    """Constant input tensors, in the dtypes the device program expects."""
    wd, wa, hvec, whd, wha = _build_weights(scaling)
    np_mm = np.float16 if IO16 else np.float32
    consts = {
        "w_d": wd.astype(np_mm),
        "w_a": wa.astype(np_mm),
        "ident": np.eye(128, dtype=NP_IO),
    }
    if HALO_MM:
        consts["w_halo_d"] = whd.astype(np_mm)
        consts["w_halo_a"] = wha.astype(np_mm)
    else:
        consts["w_hvec"] = hvec.astype(NP_IO)
    return consts


def _build(rows_per_core: int, mm_f32r: bool, repeat: int = 1, ablate: str = ""):
    key = (rows_per_core, mm_f32r, IO16, repeat, ablate)
    if key in _BUILD_CACHE:
        return _BUILD_CACHE[key]

    ngroups = rows_per_core // PG
    io_dt = F16 if IO16 else F32
    mm_dt = F16 if IO16 else (F32R if mm_f32r else F32)
    tile_blks = 8 if OCTET else 4  # input blocks per transpose tile
    tile_w = 128 * tile_blks
    ntile = NBLK // tile_blks  # transpose tiles per stream per group

    nc = bacc.Bacc("TRN2", target_bir_lowering=False, debug=False)
    d_dram = nc.dram_tensor("details", [rows_per_core, M], io_dt, kind="ExternalInput").ap()
    a_dram = nc.dram_tensor("approximation", [rows_per_core, M], io_dt, kind="ExternalInput").ap()
    wd_dram = nc.dram_tensor("w_d", [BLK, OUTW], mm_dt, kind="ExternalInput").ap()
    wa_dram = nc.dram_tensor("w_a", [BLK, OUTW], mm_dt, kind="ExternalInput").ap()
    if HALO_MM:
        whd_dram = nc.dram_tensor(
            "w_halo_d", [4 * HALO_NC, HALO_W * HALO_NC], mm_dt, kind="ExternalInput"
        ).ap()
        wha_dram = nc.dram_tensor(
            "w_halo_a", [4 * HALO_NC, HALO_W * HALO_NC], mm_dt, kind="ExternalInput"
        ).ap()
    else:
        hv_dram = nc.dram_tensor(
            "w_hvec", [128, len(HALO_POS)], io_dt, kind="ExternalInput"
        ).ap()
    id_dram = nc.dram_tensor("ident", [128, 128], io_dt, kind="ExternalInput").ap()
    out_dram = nc.dram_tensor("out", [rows_per_core, 2 * M], io_dt, kind="ExternalOutput").ap()

    store_eng = {
        "sync": nc.sync,
        "scalar": nc.scalar,
        "pool": nc.gpsimd,
    }[os.environ.get("BASS_STORE_ENG", "pool")]
    haloadd_eng = {
        "vector": nc.vector,
        "pool": nc.gpsimd,
    }[os.environ.get("BASS_HALOADD_ENG", "vector")]
    out_engs = {
        "s": nc.scalar,
        "g": nc.gpsimd,
        "v": nc.vector,
    }
    pattern = [out_engs[c] for c in OUT_PATTERN]

    with tile.TileContext(nc) as tc, ExitStack() as ctx:
        const = ctx.enter_context(tc.tile_pool(name="const", bufs=1))
        inp = ctx.enter_context(
            tc.tile_pool(name="inp", bufs=int(os.environ.get("BASS_INBUFS", "4")))
        )
        tq = ctx.enter_context(tc.tile_pool(name="tq", bufs=3))
        outp = ctx.enter_context(
            tc.tile_pool(name="outp", bufs=int(os.environ.get("BASS_OUTBUFS", "6")))
        )
        ps_t = ctx.enter_context(tc.tile_pool(name="ps_t", bufs=3, space="PSUM"))
        ps_o = ctx.enter_context(
            tc.tile_pool(name="ps_o", bufs=int(os.environ.get("BASS_PSOBUFS", "5")), space="PSUM")
        )

        def load_half(grp, hh, halves):
            r0 = grp * PG
            for st, dram in (("d", d_dram), ("a", a_dram)):
                t = inp.tile([PG, EXTW], io_dt, tag=f"in_{st}",
                             name=f"in_{st}_g{grp}h{hh}")
                if hh == 0:
                    # block 32's first 4 cols are contiguous: one DMA
                    nc.sync.dma_start(
                        t[:, 0 : HALF + 4], dram[r0 : r0 + PG, 0 : HALF + 4]
                    )
                else:
                    nc.sync.dma_start(
                        t[:, 0:HALF], dram[r0 : r0 + PG, HALF:M]
                    )
                halves[(st, hh)] = t

        def load_wrap_copies(halves):
            for st in ("d", "a"):
                # wrapped halo for the high half: row cols 0:4, already in
                # the low strip - tiny DVE copy instead of a 2nd DMA
                nc.vector.tensor_copy(
                    out=halves[(st, 1)][:, HALF : HALF + 4],
                    in_=halves[(st, 0)][:, 0:4],
                )

        def load_group_inputs(grp):
            # halves-outer order: the first octet needs d-h0 AND a-h0, so
            # both h0 strips load before either h1 strip. Wrap copies are
            # NOT emitted here: a DVE copy waiting on a future group's DMA
            # would clog the 4-deep DVE wait queue and starve the octet
            # copies; emit_group issues them right before they are needed
            # (quarter 3's halo gather).
            halves = {}
            load_half(grp, 0, halves)
            load_half(grp, 1, halves)
            return halves

        # DMA issue order for the head of the kernel: the tiny identity
        # first (warmup + all transposes need it), then the first pieces of
        # the compute-critical h0 strips (split small so the first octet's
        # transposes start ~1.7us in instead of ~6us), then the remaining
        # consts (needed by the first matmuls), then the rest.
        ident_s = const.tile([128, 128], io_dt)
        nc.sync.dma_start(ident_s[:], id_dram)

        g0_halves = None
        if repeat == 1:
            g0_halves = {}
            for st, dram in (("d", d_dram), ("a", a_dram)):
                t = inp.tile([PG, EXTW], io_dt, tag=f"in_{st}",
                             name=f"in_{st}_g0h0")
                nc.sync.dma_start(t[:, 0:1024], dram[0:PG, 0:1024])
                g0_halves[(st, 0)] = t

        wd_s = const.tile([BLK, OUTW], mm_dt)
        nc.sync.dma_start(wd_s[:], wd_dram)
        wa_s = const.tile([BLK, OUTW], mm_dt)
        nc.sync.dma_start(wa_s[:], wa_dram)
        if HALO_MM:
            whd_s = const.tile([4 * HALO_NC, HALO_W * HALO_NC], mm_dt)
            nc.sync.dma_start(whd_s[:], whd_dram)
            wha_s = const.tile([4 * HALO_NC, HALO_W * HALO_NC], mm_dt)
            nc.sync.dma_start(wha_s[:], wha_dram)
        else:
            hv_s = const.tile([128, len(HALO_POS)], io_dt)
            nc.sync.dma_start(hv_s[:], hv_dram)

        if g0_halves is not None:
            for st, dram in (("d", d_dram), ("a", a_dram)):
                t = g0_halves[(st, 0)]
                nc.sync.dma_start(t[:, 1024 : HALF + 4], dram[0:PG, 1024 : HALF + 4])
            load_half(0, 1, g0_halves)

        if os.environ.get("BASS_WARMUP", "1") == "1":
            # ~4.3us of dummy PE work at kernel start, hidden under the first
            # input DMA: trips the HAM activity window so the first real
            # transposes/matmuls run at 2.4 GHz instead of the cold 1.2 GHz.
            warm = ps_t.tile([128, 128], F32, tag="ps_t", name="warm")
            for _ in range(10):
                nc.tensor.matmul(warm[:], ident_s[:], ident_s[:], start=True,
                                 stop=True, skip_group_check=True)

        deferred_stores = []

        def emit_group(grp, halves=None):
            r0 = grp * PG
            if halves is None:
                halves = load_group_inputs(grp)

            out_halves = [
                outp.tile([PG, 2 * HALF], io_dt, tag="out", name=f"out_g{grp}h{i}")
                for i in range(2)
            ]

            if ablate == "dma":
                for hh in range(2):
                    nc.vector.tensor_copy(
                        out=out_halves[hh][:, 0:1], in_=halves[("d", hh)][:, 0:1]
                    )
                    store_eng.dma_start(
                        out_dram[r0 : r0 + PG, hh * 2 * HALF : (hh + 1) * 2 * HALF],
                        out_halves[hh][:],
                    )
                return

            tiles = {"d": [], "a": []}

            def make_tile(st, q):
                blocks = [tile_blks * q + i for i in range(tile_blks)]
                pt = ps_t.tile([128, tile_w], io_dt, tag="ps_t", name=f"pt_{st}{q}")
                for i, b in enumerate(blocks):
                    hh, off = divmod(b, NBLK_HALF)
                    nc.tensor.transpose(
                        pt[:, 128 * i : 128 * (i + 1)],
                        halves[(st, hh)][:, off * BLK : (off + 1) * BLK],
                        ident_s[:],
                    )
                qt = tq.tile([128, tile_w], mm_dt, tag=f"tq_{st}", name=f"qt_{st}{q}")
                nc.vector.tensor_copy(out=qt[:], in_=pt[:])
                tiles[st].append(qt)

            def make_chunk_pair(t):
                # chunks k=2t, 2t+1 share one PSUM bank and one copy
                po = ps_o.tile([128, 2 * OUTW], F32, tag="ps_o", name=f"po_{t}")
                for half_idx in range(2):
                    k = 2 * t + half_idx
                    q, off = divmod(k, tile_blks)
                    lhs_d = tiles["d"][q][:, off * 128 : off * 128 + 128]
                    lhs_a = tiles["a"][q][:, off * 128 : off * 128 + 128]
                    sl = po[:, half_idx * OUTW : (half_idx + 1) * OUTW]
                    nc.tensor.matmul(sl, lhs_d, wd_s[:], start=True, stop=False,
                                     skip_group_check=True)
                    nc.tensor.matmul(sl, lhs_a, wa_s[:], start=False, stop=True,
                                     skip_group_check=True)
                hh, tt = divmod(t, NBLK_HALF // 2)
                eng = pattern[t % len(pattern)]
                dst = out_halves[hh][:, tt * 2 * OUTW : (tt + 1) * 2 * OUTW]
                if eng is nc.scalar:
                    eng.copy(out=dst, in_=po[:])
                else:
                    eng.tensor_copy(out=dst, in_=po[:])

            def emit_halo_mm(c0, nch):
                # chunk range [c0, c0+nch) within one half: gather next-block
                # cols 0:4, transpose, one [4*nch x 7*nch] matmul per stream
                # accumulated in PSUM, one strided DVE add into outcols
                # 249..255. The halo weight matrix is block-diagonal per
                # chunk, so its leading [4*nch, 7*nch] slice serves any
                # aligned sub-range.
                hh, lc0 = divmod(c0, NBLK_HALF)
                hp = ps_o.tile([128, HALO_W * nch], F32, tag="ps_o",
                               name=f"hp_g{grp}c{c0}")
                for sti, (st, w_s) in enumerate((("d", whd_s), ("a", wha_s))):
                    x3 = halves[(st, hh)][:].rearrange("p (c w) -> p c w", w=BLK)
                    gather = x3[:, lc0 + 1 : lc0 + nch + 1, 0:4]
                    # walrus requires a single-free-dim stationary: compact
                    # the strided gather via DVE before the PE transpose
                    gx = tq.tile([128, 4 * nch], io_dt, tag="tq_g",
                                 name=f"gx_{st}_g{grp}c{c0}")
                    gx3 = gx[:].rearrange("p (c w) -> p c w", w=4)
                    nc.vector.tensor_copy(out=gx3, in_=gather)
                    xt = ps_t.tile([4 * nch, 128], io_dt, tag="ps_t",
                                   name=f"xt_{st}_g{grp}c{c0}")
                    nc.tensor.transpose(xt[:], gx[:], ident_s[:])
                    xs = tq.tile([4 * nch, 128], mm_dt, tag="tq_h",
                                 name=f"xs_{st}_g{grp}c{c0}")
                    nc.vector.tensor_copy(out=xs[:], in_=xt[:])
                    nc.tensor.matmul(hp[:], xs[:],
                                     w_s[0 : 4 * nch, 0 : HALO_W * nch],
                                     start=(sti == 0), stop=(sti == 1),
                                     skip_group_check=True)
                oh3 = out_halves[hh][:].rearrange("p (c w) -> p c w", w=OUTW)
                o = oh3[:, lc0 : lc0 + nch, HALO_NLO:OUTW]
                hp3 = hp[:].rearrange("p (c w) -> p c w", w=HALO_W)
                haloadd_eng.tensor_tensor(
                    out=o, in0=hp3[:, :, :], in1=o, op=mybir.AluOpType.add
                )

            def store_range(c0, nch):
                hh, lc0 = divmod(c0, NBLK_HALF)
                dram_sl = out_dram[r0 : r0 + PG, c0 * OUTW : (c0 + nch) * OUTW]
                sbuf_sl = out_halves[hh][:, lc0 * OUTW : (lc0 + nch) * OUTW]
                if grp < ngroups - 1 and hh == 1:
                    # hold mid-kernel h1 stores in reserve: they are re-issued
                    # during the final group's PE-paced drain, where the DMA
                    # engines would otherwise idle between stores
                    deferred_stores.append((dram_sl, sbuf_sl))
                else:
                    store_eng.dma_start(dram_sl, sbuf_sl)

            def flush_deferred(keep=0):
                # the sync queue is idle once all loads are issued; deferred
                # stores have no unmet deps, so they stream immediately and
                # keep the DMA engines fed through the PE-paced drain
                while len(deferred_stores) > keep:
                    dram_sl, sbuf_sl = deferred_stores.pop(0)
                    nc.sync.dma_start(dram_sl, sbuf_sl)

            def emit_halo_stt(hh):
                oh3 = out_halves[hh][:].rearrange("p (c w) -> p c w", w=OUTW)
                for i, (sti, kp, n, tap) in enumerate(HALO_POS):
                    st = "d" if sti == 0 else "a"
                    x3 = halves[(st, hh)][:].rearrange("p (c w) -> p c w", w=BLK)
                    o = oh3[:, :, n : n + 1]
                    nc.vector.scalar_tensor_tensor(
                        out=o,
                        in0=x3[:, 1 : NBLK_HALF + 1, kp : kp + 1],
                        scalar=hv_s[:, i : i + 1],
                        in1=o,
                        op0=mybir.AluOpType.mult,
                        op1=mybir.AluOpType.add,
                    )

            npairs_tile = tile_blks // 2
            chunks_done = 0
            chunks_flushed = 0
            wrap_emitted = False
            step = HALO_NC
            last_grp = grp == ngroups - 1

            def flush_ranges(limit):
                nonlocal chunks_flushed
                while chunks_flushed + step <= limit:
                    emit_halo_mm(chunks_flushed, step)
                    store_range(chunks_flushed, step)
                    chunks_flushed += step

            if last_grp and HALO_MM:
                flush_deferred()

            for q in range(ntile):
                if q == ntile - 1:
                    # the last range's halo gather reads the wrapped columns;
                    # emit the wrap copies now that both strips have long
                    # been resident (emitting them at load time would clog
                    # the DVE wait queue for microseconds)
                    load_wrap_copies(halves)
                    wrap_emitted = True
                make_tile("d", q)
                make_tile("a", q)
                for t in range(npairs_tile * q, npairs_tile * (q + 1)):
                    make_chunk_pair(t)
                chunks_done += 2 * npairs_tile
                if HALO_MM:
                    # one-octet lag: a range's halo+store are emitted after
                    # the NEXT octet's tiles so the halo add (waiting on the
                    # range's chunk copies) doesn't clog the DVE wait queue
                    # ahead of the octet copies that feed PE. No lag on the
                    # final group - there the stores ARE the critical path.
                    lag = 0 if grp == ngroups - 1 else 2 * npairs_tile
                    flush_ranges(chunks_done - lag)
            if HALO_MM:
                flush_ranges(NBLK)
                if last_grp:
                    flush_deferred()

            if not HALO_MM:
                if not wrap_emitted:
                    load_wrap_copies(halves)
                for hh in range(2):
                    emit_halo_stt(hh)
                    store_eng.dma_start(
                        out_dram[r0 : r0 + PG, hh * 2 * HALF : (hh + 1) * 2 * HALF],
                        out_halves[hh][:],
                    )

        if repeat > 1:
            with tc.For_i(0, repeat, 1):
                for grp in range(ngroups):
                    emit_group(grp)
        else:
            for grp in range(ngroups):
                emit_group(grp, g0_halves if grp == 0 else None)

    nc.compile()
    _BUILD_CACHE[key] = nc
    return nc


def _run(details, approximation, scaling, rows_per_core, core_ids, mm_f32r, **kw):
    consts = host_consts(scaling)
    nc = _build(rows_per_core, mm_f32r)
    d_io = np.ascontiguousarray(details, dtype=NP_IO)
    a_io = np.ascontiguousarray(approximation, dtype=NP_IO)
    in_maps = []
    for c in core_ids:
        r0 = c * rows_per_core
        in_maps.append(
            {
                "details": d_io[r0 : r0 + rows_per_core],
                "approximation": a_io[r0 : r0 + rows_per_core],
                **consts,
            }
        )
    res = run_bass_kernel_spmd(nc, in_maps, core_ids=list(range(len(core_ids))), **kw)
    out = np.concatenate([res.results[i]["out"] for i in range(len(core_ids))], axis=0)
    return out, res


def kernel(details, approximation, scaling):
    details = np.asarray(details, dtype=np.float32)
    approximation = np.asarray(approximation, dtype=np.float32)
    scaling = np.asarray(scaling, dtype=np.float32)
    rows_per_core = details.shape[0] // N_CORES
    out, _ = _run(
        details, approximation, scaling, rows_per_core, list(range(N_CORES)),
        MM_F32R,
    )
    return np.ascontiguousarray(out, dtype=np.float32)


# revision 55
# speedup vs baseline: 1.0632x; 1.0632x over previous
"""Trainium2 Bass kernel for nn_BackwardTransformLayer (inverse DWT synthesis step).

Math: out[r, 2j+s] = sum_{p=0..3} g[2p+s]*d[r,(j+p+s')%M] + h[2p+s]*a[r,...]
  (g = flip(scaling) with odd idx negated; h = scaling; even outputs use
   shifts 0..3 of taps g[0,2,4,6], odd outputs shifts 1..4 of g[1,3,5,7])

Strategy (8 cores data-parallel over rows, 512 rows/core):
  - fp16 I/O: inputs are quantized to fp16 on the host and the output is
    written fp16, halving HBM traffic (f32 I/O saturates the ~358 GB/s/core
    DMA ceiling at ~183us; fp16 floor is ~94us). Quantization noise is
    ~3e-4 fro rel err, far below the 2e-2 gate. fp16 also halves PE
    transpose cost (1 cyc/row vs 2 for fp32).
  - The polyphase stencil along columns is a banded linear operator: for each
    128-column input block k, out[:, 256k:256k+256] = d_blk @ W_d + a_blk @ W_a
    plus a "halo" contribution from the first 4 columns of block k+1
    (circularly wrapped) hitting output columns 249..255 of the chunk.
  - TensorE transposes input blocks (8 blocks -> one [128,1024] PSUM bank),
    DVE copies them to SBUF, and TensorE computes the banded products with
    stationary = dT slice and moving = W[incol, outcol]; results land
    naturally oriented [row, outcol] in PSUM.
  - Halo: the 4 wrapped columns per block are gathered (strided stationary)
    into one PE transpose per (stream, half), then a single [128k x 224]
    matmul per half accumulates both streams' halo contributions; one strided
    DVE add folds them into output columns 249..255 of every chunk. (The old
    path - 64 tiny strided MACs per group - is kept behind BASS_HALO=stt.)
  - Output chunk-pair copies PSUM->SBUF rotate ACT/ACT/DVE/ACT (only ACT and
    DVE may read PSUM on HW; GPSIMD raises a BIR verifier error).
  - Scheduling (sim-driven): output stores issue on the GPSIMD HWDGE queue
    right after the DVE halo add they depend on (in-order queues: a store
    waiting at an engine's queue head must not block compute or loads);
    input loads keep the sync queue to themselves; h1 stores of groups 0..2
    are DEFERRED and re-issued on the then-idle sync queue during the final
    group so the DMA engines stay fed through the PE-paced drain; group 0's
    loads are issued as d/a-interleaved pieces sized so each octet's columns
    land just before PE demands them; the PE stream is software-pipelined
    (octet q+1's transposes emitted before octet q's matmuls, hiding the DVE
    copy latency); the wrapped first-4-columns are copied SBUF->SBUF by DVE
    late in each group (emitting at load time would clog the 4-deep DVE
    wait queue).
  - TimelineSim (simprof.py) estimate: ~105us; per-core busy: DMA 94us
    (33.5 MB at ~360 GB/s, the roofline), PE 86us, ACT ~65us, DVE ~65us.

Env:
  BASS_IO16=1 (default) fp16 DRAM I/O + fp16 matmuls; =0 f32 I/O (legacy).
  BASS_MM_F32R=1 (default) legacy-path matmuls in float32r; =0 exact fp32.
  BASS_HALO=mm|stt (default mm) halo via batched matmul or strided MACs.
  BASS_OUT_PATTERN=ssvs  output-copy engine rotation: s=ACT v=DVE.
  BASS_OCTET=1 (default) 8-block [128,1024] transpose tiles; =0 quads.
  BASS_STORE_ENG=pool|sync|scalar (default pool) store HWDGE queue.
"""

import os
import sys
from contextlib import ExitStack

import numpy as np

sys.path.insert(0, "/opt/trn_rl_repo")

import concourse.bass as bass  # noqa: E402
import concourse.mybir as mybir  # noqa: E402
import concourse.tile as tile  # noqa: E402
from concourse import bacc  # noqa: E402
from concourse.bass_utils import run_bass_kernel_spmd  # noqa: E402

N_CORES = 8
N_ROWS = 4096
M = 8192  # input columns per row
PG = 128  # rows per group (partition dim)
BLK = 128  # input columns per block
OUTW = 2 * BLK  # output columns per chunk
HALF = M // 2  # input columns per half-strip
NBLK_HALF = HALF // BLK  # 32 blocks per half-strip
NBLK = M // BLK  # 64 blocks
EXTW = HALF + BLK  # extended strip width (one extra block; 4 cols used)
F32 = mybir.dt.float32
F32R = mybir.dt.float32r
F16 = mybir.dt.float16

IO16 = os.environ.get("BASS_IO16", "1") == "1"
MM_F32R = os.environ.get("BASS_MM_F32R", "1") == "1"
HALO_MM = os.environ.get("BASS_HALO", "mm") == "mm"
# chunk-pair copy engines: only ACT ('s') and DVE ('v') may read PSUM on
# real HW (GPSIMD raises "cannot access PSUM" in the BIR verifier)
OUT_PATTERN = os.environ.get("BASS_OUT_PATTERN", "ssvs")
OCTET = os.environ.get("BASS_OCTET", "1") == "1"

NP_IO = np.float16 if IO16 else np.float32

_BUILD_CACHE = {}


def _halo_positions():
    """Static (stream, kp, n, tap) positions of halo coefficients.

    Chunk outcol n (0..255) gets a contribution coeff[tap] * x[:, 128*(k+1)+kp]
    from the next block's first 4 columns.
    """
    pos = []
    for sti in range(2):  # 0 = details (g), 1 = approximation (h)
        for v in range(128):
            for s in range(4):
                kp = v + s - 128
                if 0 <= kp <= 3:
                    pos.append((sti, kp, 2 * v, 2 * s))
                kp2 = v + 1 + s - 128
                if 0 <= kp2 <= 3:
                    pos.append((sti, kp2, 2 * v + 1, 2 * s + 1))
    return pos


HALO_POS = _halo_positions()  # 32 entries
HALO_NLO = 249  # lowest chunk outcol touched by halo
HALO_W = 256 - HALO_NLO  # 7 consecutive outcols per chunk
HALO_NC = 16  # chunks per halo batch (quarter of a group)


def _build_weights(scaling: np.ndarray):
    h = np.asarray(scaling, dtype=np.float32)
    g = h[::-1].copy()
    g[1::2] *= -1.0

    def build_main(f):
        W = np.zeros((BLK, OUTW), np.float32)
        for k in range(BLK):
            for v in range(BLK):
                s = k - v
                if 0 <= s <= 3:
                    W[k, 2 * v] = f[2 * s]
                s = k - v - 1
                if 0 <= s <= 3:
                    W[k, 2 * v + 1] = f[2 * s + 1]
        return W

    hvec = np.zeros((128, len(HALO_POS)), np.float32)
    for i, (sti, kp, n, tap) in enumerate(HALO_POS):
        hvec[:, i] = (g if sti == 0 else h)[tap]

    def build_halo(f, sti_sel):
        # row = 4c + kp (transposed gather layout), col = 7c + (n - 249);
        # one quarter-group (16 chunks) worth - identical for every quarter
        W = np.zeros((4 * HALO_NC, HALO_W * HALO_NC), np.float32)
        for sti, kp, n, tap in HALO_POS:
            if sti != sti_sel:
                continue
            for c in range(HALO_NC):
                W[4 * c + kp, HALO_W * c + (n - HALO_NLO)] = f[tap]
        return W

    return build_main(g), build_main(h), hvec, build_halo(g, 0), build_halo(h, 1)


def host_consts(scaling: np.ndarray) -> dict:
    """Constant input tensors, in the dtypes the device program expects."""
    wd, wa, hvec, whd, wha = _build_weights(scaling)
    np_mm = np.float16 if IO16 else np.float32
    consts = {
        "w_d": wd.astype(np_mm),
        "w_a": wa.astype(np_mm),
        "ident": np.eye(128, dtype=NP_IO),
    }
    if HALO_MM:
        consts["w_halo_d"] = whd.astype(np_mm)
        consts["w_halo_a"] = wha.astype(np_mm)
    else:
        consts["w_hvec"] = hvec.astype(NP_IO)
    return consts


def _build(rows_per_core: int, mm_f32r: bool, repeat: int = 1, ablate: str = ""):
    key = (rows_per_core, mm_f32r, IO16, repeat, ablate)
    if key in _BUILD_CACHE:
        return _BUILD_CACHE[key]

    ngroups = rows_per_core // PG
    io_dt = F16 if IO16 else F32
    mm_dt = F16 if IO16 else (F32R if mm_f32r else F32)
    tile_blks = 8 if OCTET else 4  # input blocks per transpose tile
    tile_w = 128 * tile_blks
    ntile = NBLK // tile_blks  # transpose tiles per stream per group

    nc = bacc.Bacc("TRN2", target_bir_lowering=False, debug=False)
    d_dram = nc.dram_tensor("details", [rows_per_core, M], io_dt, kind="ExternalInput").ap()
    a_dram = nc.dram_tensor("approximation", [rows_per_core, M], io_dt, kind="ExternalInput").ap()
    wd_dram = nc.dram_tensor("w_d", [BLK, OUTW], mm_dt, kind="ExternalInput").ap()
    wa_dram = nc.dram_tensor("w_a", [BLK, OUTW], mm_dt, kind="ExternalInput").ap()
    if HALO_MM:
        whd_dram = nc.dram_tensor(
            "w_halo_d", [4 * HALO_NC, HALO_W * HALO_NC], mm_dt, kind="ExternalInput"
        ).ap()
        wha_dram = nc.dram_tensor(
            "w_halo_a", [4 * HALO_NC, HALO_W * HALO_NC], mm_dt, kind="ExternalInput"
        ).ap()
    else:
        hv_dram = nc.dram_tensor(
            "w_hvec", [128, len(HALO_POS)], io_dt, kind="ExternalInput"
        ).ap()
    id_dram = nc.dram_tensor("ident", [128, 128], io_dt, kind="ExternalInput").ap()
    out_dram = nc.dram_tensor("out", [rows_per_core, 2 * M], io_dt, kind="ExternalOutput").ap()

    store_eng = {
        "sync": nc.sync,
        "scalar": nc.scalar,
        "pool": nc.gpsimd,
    }[os.environ.get("BASS_STORE_ENG", "pool")]
    haloadd_eng = {
        "vector": nc.vector,
        "pool": nc.gpsimd,
    }[os.environ.get("BASS_HALOADD_ENG", "vector")]
    out_engs = {
        "s": nc.scalar,
        "g": nc.gpsimd,
        "v": nc.vector,
    }
    pattern = [out_engs[c] for c in OUT_PATTERN]

    with tile.TileContext(nc) as tc, ExitStack() as ctx:
        const = ctx.enter_context(tc.tile_pool(name="const", bufs=1))
        inp = ctx.enter_context(
            tc.tile_pool(name="inp", bufs=int(os.environ.get("BASS_INBUFS", "5")))
        )
        tq = ctx.enter_context(tc.tile_pool(name="tq", bufs=3))
        outp = ctx.enter_context(
            tc.tile_pool(name="outp", bufs=int(os.environ.get("BASS_OUTBUFS", "5")))
        )
        ps_t = ctx.enter_context(tc.tile_pool(name="ps_t", bufs=3, space="PSUM"))
        ps_o = ctx.enter_context(
            tc.tile_pool(name="ps_o", bufs=int(os.environ.get("BASS_PSOBUFS", "5")), space="PSUM")
        )

        def load_half(grp, hh, halves):
            r0 = grp * PG
            for st, dram in (("d", d_dram), ("a", a_dram)):
                t = inp.tile([PG, EXTW], io_dt, tag=f"in_{st}",
                             name=f"in_{st}_g{grp}h{hh}")
                if hh == 0:
                    # block 32's first 4 cols are contiguous: one DMA
                    nc.sync.dma_start(
                        t[:, 0 : HALF + 4], dram[r0 : r0 + PG, 0 : HALF + 4]
                    )
                else:
                    nc.sync.dma_start(
                        t[:, 0:HALF], dram[r0 : r0 + PG, HALF:M]
                    )
                halves[(st, hh)] = t

        def load_wrap_copies(halves):
            for st in ("d", "a"):
                # wrapped halo for the high half: row cols 0:4, already in
                # the low strip - tiny DVE copy instead of a 2nd DMA
                nc.vector.tensor_copy(
                    out=halves[(st, 1)][:, HALF : HALF + 4],
                    in_=halves[(st, 0)][:, 0:4],
                )

        def load_group_inputs(grp):
            # halves-outer order: the first octet needs d-h0 AND a-h0, so
            # both h0 strips load before either h1 strip. Wrap copies are
            # NOT emitted here: a DVE copy waiting on a future group's DMA
            # would clog the 4-deep DVE wait queue and starve the octet
            # copies; emit_group issues them right before they are needed
            # (quarter 3's halo gather).
            halves = {}
            load_half(grp, 0, halves)
            load_half(grp, 1, halves)
            return halves

        # DMA issue order for the head of the kernel: the tiny identity
        # first (warmup + all transposes need it), then the first pieces of
        # the compute-critical h0 strips (split small so the first octet's
        # transposes start ~2us in instead of ~6us), then the remaining
        # consts (needed by the first matmuls), then the rest of group 0 in
        # d/a-interleaved pieces sized so each octet's columns land just
        # before PE demands them (~2.1us per octet).
        ident_s = const.tile([128, 128], io_dt)
        nc.sync.dma_start(ident_s[:], id_dram)

        g0_halves = None
        if repeat == 1:
            g0_halves = {}
            for st, dram in (("d", d_dram), ("a", a_dram)):
                t = inp.tile([PG, EXTW], io_dt, tag=f"in_{st}",
                             name=f"in_{st}_g0h0")
                nc.sync.dma_start(t[:, 0:1024], dram[0:PG, 0:1024])
                g0_halves[(st, 0)] = t

        wd_s = const.tile([BLK, OUTW], mm_dt)
        nc.sync.dma_start(wd_s[:], wd_dram)
        wa_s = const.tile([BLK, OUTW], mm_dt)
        nc.sync.dma_start(wa_s[:], wa_dram)
        if HALO_MM:
            whd_s = const.tile([4 * HALO_NC, HALO_W * HALO_NC], mm_dt)
            nc.sync.dma_start(whd_s[:], whd_dram)
            wha_s = const.tile([4 * HALO_NC, HALO_W * HALO_NC], mm_dt)
            nc.sync.dma_start(wha_s[:], wha_dram)
        else:
            hv_s = const.tile([128, len(HALO_POS)], io_dt)
            nc.sync.dma_start(hv_s[:], hv_dram)

        if g0_halves is not None:
            # h0 remainder in d/a-interleaved pieces
            for lo, hi in ((1024, 2560), (2560, HALF + 4)):
                for st, dram in (("d", d_dram), ("a", a_dram)):
                    t = g0_halves[(st, 0)]
                    nc.sync.dma_start(t[:, lo:hi], dram[0:PG, lo:hi])
            # h1 strips, also interleaved in two pieces each
            for st, dram in (("d", d_dram), ("a", a_dram)):
                t = inp.tile([PG, EXTW], io_dt, tag=f"in_{st}",
                             name=f"in_{st}_g0h1")
                g0_halves[(st, 1)] = t
            for lo, hi in ((0, 2048), (2048, HALF)):
                for st, dram in (("d", d_dram), ("a", a_dram)):
                    nc.sync.dma_start(
                        g0_halves[(st, 1)][:, lo:hi],
                        dram[0:PG, HALF + lo : HALF + hi],
                    )

        if os.environ.get("BASS_WARMUP", "1") == "1":
            # ~4.3us of dummy PE work at kernel start, hidden under the first
            # input DMA: trips the HAM activity window so the first real
            # transposes/matmuls run at 2.4 GHz instead of the cold 1.2 GHz.
            warm = ps_t.tile([128, 128], F32, tag="ps_t", name="warm")
            for _ in range(10):
                nc.tensor.matmul(warm[:], ident_s[:], ident_s[:], start=True,
                                 stop=True, skip_group_check=True)

        deferred_stores = []

        def emit_group(grp, halves=None):
            r0 = grp * PG
            if halves is None:
                halves = load_group_inputs(grp)

            out_halves = [
                outp.tile([PG, 2 * HALF], io_dt, tag="out", name=f"out_g{grp}h{i}")
                for i in range(2)
            ]

            if ablate == "dma":
                for hh in range(2):
                    nc.vector.tensor_copy(
                        out=out_halves[hh][:, 0:1], in_=halves[("d", hh)][:, 0:1]
                    )
                    store_eng.dma_start(
                        out_dram[r0 : r0 + PG, hh * 2 * HALF : (hh + 1) * 2 * HALF],
                        out_halves[hh][:],
                    )
                return

            tiles = {"d": [], "a": []}

            def make_tile(st, q):
                blocks = [tile_blks * q + i for i in range(tile_blks)]
                pt = ps_t.tile([128, tile_w], io_dt, tag="ps_t", name=f"pt_{st}{q}")
                for i, b in enumerate(blocks):
                    hh, off = divmod(b, NBLK_HALF)
                    nc.tensor.transpose(
                        pt[:, 128 * i : 128 * (i + 1)],
                        halves[(st, hh)][:, off * BLK : (off + 1) * BLK],
                        ident_s[:],
                    )
                qt = tq.tile([128, tile_w], mm_dt, tag=f"tq_{st}", name=f"qt_{st}{q}")
                nc.vector.tensor_copy(out=qt[:], in_=pt[:])
                tiles[st].append(qt)

            def make_chunk_pair(t):
                # chunks k=2t, 2t+1 share one PSUM bank and one copy
                po = ps_o.tile([128, 2 * OUTW], F32, tag="ps_o", name=f"po_{t}")
                for half_idx in range(2):
                    k = 2 * t + half_idx
                    q, off = divmod(k, tile_blks)
                    lhs_d = tiles["d"][q][:, off * 128 : off * 128 + 128]
                    lhs_a = tiles["a"][q][:, off * 128 : off * 128 + 128]
                    sl = po[:, half_idx * OUTW : (half_idx + 1) * OUTW]
                    nc.tensor.matmul(sl, lhs_d, wd_s[:], start=True, stop=False,
                                     skip_group_check=True)
                    nc.tensor.matmul(sl, lhs_a, wa_s[:], start=False, stop=True,
                                     skip_group_check=True)
                hh, tt = divmod(t, NBLK_HALF // 2)
                eng = pattern[t % len(pattern)]
                dst = out_halves[hh][:, tt * 2 * OUTW : (tt + 1) * 2 * OUTW]
                if eng is nc.scalar:
                    eng.copy(out=dst, in_=po[:])
                else:
                    eng.tensor_copy(out=dst, in_=po[:])

            def emit_halo_mm(c0, nch):
                # chunk range [c0, c0+nch) within one half: gather next-block
                # cols 0:4, transpose, one [4*nch x 7*nch] matmul per stream
                # accumulated in PSUM, one strided DVE add into outcols
                # 249..255. The halo weight matrix is block-diagonal per
                # chunk, so its leading [4*nch, 7*nch] slice serves any
                # aligned sub-range.
                hh, lc0 = divmod(c0, NBLK_HALF)
                hp = ps_o.tile([128, HALO_W * nch], F32, tag="ps_o",
                               name=f"hp_g{grp}c{c0}")
                for sti, (st, w_s) in enumerate((("d", whd_s), ("a", wha_s))):
                    x3 = halves[(st, hh)][:].rearrange("p (c w) -> p c w", w=BLK)
                    gather = x3[:, lc0 + 1 : lc0 + nch + 1, 0:4]
                    # walrus requires a single-free-dim stationary: compact
                    # the strided gather via DVE before the PE transpose
                    gx = tq.tile([128, 4 * nch], io_dt, tag="tq_g",
                                 name=f"gx_{st}_g{grp}c{c0}")
                    gx3 = gx[:].rearrange("p (c w) -> p c w", w=4)
                    nc.vector.tensor_copy(out=gx3, in_=gather)
                    xt = ps_t.tile([4 * nch, 128], io_dt, tag="ps_t",
                                   name=f"xt_{st}_g{grp}c{c0}")
                    nc.tensor.transpose(xt[:], gx[:], ident_s[:])
                    xs = tq.tile([4 * nch, 128], mm_dt, tag="tq_h",
                                 name=f"xs_{st}_g{grp}c{c0}")
                    nc.vector.tensor_copy(out=xs[:], in_=xt[:])
                    nc.tensor.matmul(hp[:], xs[:],
                                     w_s[0 : 4 * nch, 0 : HALO_W * nch],
                                     start=(sti == 0), stop=(sti == 1),
                                     skip_group_check=True)
                oh3 = out_halves[hh][:].rearrange("p (c w) -> p c w", w=OUTW)
                o = oh3[:, lc0 : lc0 + nch, HALO_NLO:OUTW]
                hp3 = hp[:].rearrange("p (c w) -> p c w", w=HALO_W)
                haloadd_eng.tensor_tensor(
                    out=o, in0=hp3[:, :, :], in1=o, op=mybir.AluOpType.add
                )

            def store_range(c0, nch):
                hh, lc0 = divmod(c0, NBLK_HALF)
                dram_sl = out_dram[r0 : r0 + PG, c0 * OUTW : (c0 + nch) * OUTW]
                sbuf_sl = out_halves[hh][:, lc0 * OUTW : (lc0 + nch) * OUTW]
                defer_more = os.environ.get("BASS_DEFER", "h1") == "more"
                if (grp < ngroups - 1 and hh == 1) or (
                    defer_more and 1 <= grp < ngroups - 1 and hh == 0
                ):
                    # hold mid-kernel h1 stores in reserve: they are re-issued
                    # during the final group's PE-paced drain, where the DMA
                    # engines would otherwise idle between stores
                    deferred_stores.append((dram_sl, sbuf_sl))
                else:
                    store_eng.dma_start(dram_sl, sbuf_sl)

            def flush_deferred(keep=0):
                # the sync queue is idle once all loads are issued; deferred
                # stores have no unmet deps, so they stream immediately and
                # keep the DMA engines fed through the PE-paced drain
                while len(deferred_stores) > keep:
                    dram_sl, sbuf_sl = deferred_stores.pop(0)
                    nc.sync.dma_start(dram_sl, sbuf_sl)

            def emit_halo_stt(hh):
                oh3 = out_halves[hh][:].rearrange("p (c w) -> p c w", w=OUTW)
                for i, (sti, kp, n, tap) in enumerate(HALO_POS):
                    st = "d" if sti == 0 else "a"
                    x3 = halves[(st, hh)][:].rearrange("p (c w) -> p c w", w=BLK)
                    o = oh3[:, :, n : n + 1]
                    nc.vector.scalar_tensor_tensor(
                        out=o,
                        in0=x3[:, 1 : NBLK_HALF + 1, kp : kp + 1],
                        scalar=hv_s[:, i : i + 1],
                        in1=o,
                        op0=mybir.AluOpType.mult,
                        op1=mybir.AluOpType.add,
                    )

            npairs_tile = tile_blks // 2
            chunks_done = 0
            chunks_flushed = 0
            wrap_emitted = False
            step = HALO_NC
            last_grp = grp == ngroups - 1

            def flush_ranges(limit):
                nonlocal chunks_flushed
                while chunks_flushed + step <= limit:
                    emit_halo_mm(chunks_flushed, step)
                    store_range(chunks_flushed, step)
                    chunks_flushed += step

            if last_grp and HALO_MM:
                flush_deferred()

            for q in range(ntile):
                # software-pipelined PE stream: octet q+1's transposes are
                # emitted BEFORE octet q's matmuls, so PE fills the gap while
                # DVE copies octet q's transposed tiles to SBUF
                if q == 0:
                    make_tile("d", 0)
                    make_tile("a", 0)
                if q == ntile - 1:
                    # the last range's halo gather reads the wrapped columns;
                    # emit the wrap copies now that both strips have long
                    # been resident (emitting them at load time would clog
                    # the DVE wait queue for microseconds)
                    load_wrap_copies(halves)
                    wrap_emitted = True
                else:
                    make_tile("d", q + 1)
                    make_tile("a", q + 1)
                for t in range(npairs_tile * q, npairs_tile * (q + 1)):
                    make_chunk_pair(t)
                chunks_done += 2 * npairs_tile
                if HALO_MM:
                    # one-octet lag: a range's halo+store are emitted after
                    # the NEXT octet's tiles so the halo add (waiting on the
                    # range's chunk copies) doesn't clog the DVE wait queue
                    # ahead of the octet copies that feed PE. No lag on the
                    # final group - there the stores ARE the critical path.
                    lag = 0 if grp == ngroups - 1 else 2 * npairs_tile
                    flush_ranges(chunks_done - lag)
            if HALO_MM:
                flush_ranges(NBLK)
                if last_grp:
                    flush_deferred()

            if not HALO_MM:
                if not wrap_emitted:
                    load_wrap_copies(halves)
                for hh in range(2):
                    emit_halo_stt(hh)
                    store_eng.dma_start(
                        out_dram[r0 : r0 + PG, hh * 2 * HALF : (hh + 1) * 2 * HALF],
                        out_halves[hh][:],
                    )

        if repeat > 1:
            with tc.For_i(0, repeat, 1):
                for grp in range(ngroups):
                    emit_group(grp)
        else:
            for grp in range(ngroups):
                emit_group(grp, g0_halves if grp == 0 else None)

    nc.compile()
    _BUILD_CACHE[key] = nc
    return nc


def _run(details, approximation, scaling, rows_per_core, core_ids, mm_f32r, **kw):
    consts = host_consts(scaling)
    nc = _build(rows_per_core, mm_f32r)
    d_io = np.ascontiguousarray(details, dtype=NP_IO)
    a_io = np.ascontiguousarray(approximation, dtype=NP_IO)
    in_maps = []
    for c in core_ids:
        r0 = c * rows_per_core
        in_maps.append(
            {
                "details": d_io[r0 : r0 + rows_per_core],
                "approximation": a_io[r0 : r0 + rows_per_core],
                **consts,
            }
        )
    res = run_bass_kernel_spmd(nc, in_maps, core_ids=list(range(len(core_ids))), **kw)
    out = np.concatenate([res.results[i]["out"] for i in range(len(core_ids))], axis=0)
    return out, res


def kernel(details, approximation, scaling):
    details = np.asarray(details, dtype=np.float32)
    approximation = np.asarray(approximation, dtype=np.float32)
    scaling = np.asarray(scaling, dtype=np.float32)
    rows_per_core = details.shape[0] // N_CORES
    out, _ = _run(
        details, approximation, scaling, rows_per_core, list(range(N_CORES)),
        MM_F32R,
    )
    return np.ascontiguousarray(out, dtype=np.float32)
